# revision 6
# baseline (speedup 1.0000x reference)
"""BandSplitModule Trainium2 kernel.

Math (per batch element b, band i with band rows xb [t=512, feat]):
    mu, var = stats over feat;  a = rsqrt(var+eps)
    z = ((xb-mu)*a*g + lnb) @ W.T + bias
      = a * (xb @ Wg.T + s (x) negmu + bf (x) inva)
    with Wg = W*g, s_j = sum_k Wg[j,k], bf = W@lnb + bias,
         negmu = -mu, inva = sqrt(var+eps)  (so a*inva = 1).

Device plan per core (1 batch element per core, 8 cores):
  - x lands as 33 "natural" [128,512] f32r tiles (rows = (c,freq), cols = t).
  - stats: one indicator matmul per tile accumulates per-band sums into
    PSUM (x into bank S, x^2 into bank Q); postprocess gives negmu, inva, a.
  - x is repacked (SWDGE cast-DMA) into per-band K-chunk tiles (bf16) with
    2 aug rows [negmu; inva] scattered in; weights lhsT chunks carry [s; bf].
  - main matmuls accumulate psum_y [emb=128, t=512] per band (all K-chunks
    at partition base 0).
  - psum_y -> (ACT copy, bf16) -> 4 PE transposes -> psum_zT [t,(c,emb)]
    -> one DVE multiply by a^T (free-broadcast AP) -> zsb bf16
    -> SWDGE cast-DMA to fp32 HBM in [t, emb] layout.
"""

import sys
import json

sys.path.insert(0, "/opt/trn_rl_repo")

import numpy as np
import ml_dtypes

# ----------------------------------------------------------------------------
# Problem constants (hardcoded; kernel.py must be self-contained)
# ----------------------------------------------------------------------------
WIDTHS = [16] * 20 + [32] * 12 + [64] * 5 + [1]
EDGES = np.concatenate([[0], np.cumsum(WIDTHS)]).astype(int)
NBANDS = len(WIDTHS)          # 38
NFREQ = int(EDGES[-1])        # 1025
EMB = 128
NT = 512                      # time steps
NB = 8                        # batch
NCORES = 8
EPS = 1e-5
NROWS = 4 * NFREQ             # 4100 natural rows (c, f)
NTILES = (NROWS + 127) // 128  # 33 packed tiles


def _chunk_splits(feat):
    if feat <= 126:
        return [(0, feat)]
    if feat == 128:
        return [(0, 96), (96, 128)]
    if feat == 256:
        return [(0, 128), (128, 224), (224, 256)]
    raise ValueError(feat)


# Per-band chunk table: list of (k0, k1, is_last)
BAND_CHUNKS = []
for b in range(NBANDS):
    feat = 4 * WIDTHS[b]
    sp = _chunk_splits(feat)
    BAND_CHUNKS.append([(k0, k1, ci == len(sp) - 1) for ci, (k0, k1) in enumerate(sp)])

# band of each frequency bin
BAND_OF = np.zeros(NFREQ, dtype=int)
for b in range(NBANDS):
    BAND_OF[EDGES[b]:EDGES[b + 1]] = b


def _repack_segments(b, k0, k1):
    """Source segments in natural-row space for band b's chunk [k0,k1).

    Returns list of (tile_idx, src_part, dst_part, length). Natural row of
    band-local feature k (k = cr*bw + df) is cr*1025 + fs + df.
    """
    bw = WIDTHS[b]
    fs = int(EDGES[b])
    segs = []
    for cr in range(4):
        lo = max(k0, cr * bw)
        hi = min(k1, (cr + 1) * bw)
        if lo >= hi:
            continue
        n0 = cr * NFREQ + fs + (lo - cr * bw)
        dst = lo - k0
        left = hi - lo
        while left > 0:
            t = n0 // 128
            sp = n0 % 128
            ln = min(left, 128 - sp)
            segs.append((t, sp, dst, ln))
            n0 += ln
            dst += ln
            left -= ln
    return segs


# ----------------------------------------------------------------------------
# Walrus workaround: this container's walrus accepts only ONE sync-wait per
# instruction. Split multi-wait instructions into single-wait NoOps at the
# BIR-JSON level right before compile.
# ----------------------------------------------------------------------------
def _split_waits(j):
    n = 0
    for fn in j.get("functions", []):
        for blk in fn.get("blocks", []):
            new = []
            for inst in blk.get("instructions", []):
                si = inst.get("sync_info")
                waits = (si or {}).get("on_wait") or []
                plain = [w for w in waits if w.get("wait_reg") is None]
                if len(waits) > 1 and len(plain) == len(waits):
                    for k, w in enumerate(waits[:-1]):
                        new.append({
                            "name": f"{inst['name']}.w{k}",
                            "opcode": "NoOp",
                            "engine": inst["engine"],
                            "ins": [],
                            "outs": [],
                            "sync_info": {"on_wait": [w], "on_update": []},
                        })
                        n += 1
                    si["on_wait"] = [waits[-1]]
                new.append(inst)
            blk["instructions"] = new
    return n


def _install_birpatch():
    import concourse.bass_utils as bu
    import concourse.bass2jax as b2j

    if getattr(bu, "_birpatch_installed", False):
        return
    orig = bu.compile_bir_kernel

    def patched(bir_json, tmpdir, neff_name="file.neff"):
        if isinstance(bir_json, str):
            bir_json = bir_json.encode()
        j = json.loads(bir_json)
        _split_waits(j)
        return orig(json.dumps(j).encode(), tmpdir, neff_name=neff_name)

    bu.compile_bir_kernel = patched
    bu._birpatch_installed = True
    if getattr(b2j, "compile_bir_kernel", None) is not None:
        b2j.compile_bir_kernel = patched


# ----------------------------------------------------------------------------
# Bass kernel construction (built once per process)
# ----------------------------------------------------------------------------
_NC = None


def _build_nc():
    import concourse.bass as bass
    import concourse.tile as tile
    import concourse.mybir as mybir

    F32R = mybir.dt.float32r
    BF16 = mybir.dt.bfloat16
    FP32 = mybir.dt.float32

    nc = bass.Bass("TRN2", target_bir_lowering=False, debug=False,
                   num_devices=NCORES)

    x_d = nc.dram_tensor("x", [4, NFREQ, NT], FP32, kind="ExternalInput")
    wt_rows = NROWS + 2 * NBANDS
    wt_d = nc.dram_tensor("wt", [wt_rows, EMB], BF16, kind="ExternalInput")
    ind_d = nc.dram_tensor("ind", [NTILES * 128, NBANDS], FP32,
                           kind="ExternalInput")
    idb_d = nc.dram_tensor("identb", [128, 128], BF16, kind="ExternalInput")
    idf_d = nc.dram_tensor("identf", [128, 128], FP32, kind="ExternalInput")
    cst_d = nc.dram_tensor("cst", [NBANDS, 2], FP32, kind="ExternalInput")
    wt4_d = nc.dram_tensor("wt4", [6, EMB], FP32, kind="ExternalInput")
    one4_d = nc.dram_tensor("one4", [4, 1], FP32, kind="ExternalInput")
    out_d = nc.dram_tensor("out", [NBANDS, NT, EMB], FP32,
                           kind="ExternalOutput")

    x_flat = x_d[:, :, :]  # AP over [4, 1025, 512]

    # chunk weight row offsets in wt_d
    w_offs = []
    r = 0
    for b in range(NBANDS):
        offs = []
        for (k0, k1, last) in BAND_CHUNKS[b]:
            rows = (k1 - k0) + (2 if last else 0)
            offs.append((r, rows))
            r += rows
        w_offs.append(offs)
    assert r == wt_rows

    with tile.TileContext(nc) as tc:
        from contextlib import ExitStack
        with ExitStack() as ctx:
            consts = ctx.enter_context(tc.tile_pool(name="consts", bufs=1))
            xpool = ctx.enter_context(tc.tile_pool(name="xpool", bufs=NTILES))
            x2pool = ctx.enter_context(tc.tile_pool(name="x2p", bufs=4))
            indpool = ctx.enter_context(tc.tile_pool(name="indp", bufs=NTILES))
            wpool = ctx.enter_context(tc.tile_pool(name="wp", bufs=60))
            chpool = ctx.enter_context(tc.tile_pool(name="chp", bufs=60))
            stat = ctx.enter_context(tc.tile_pool(name="stat", bufs=1))
            ybfp = ctx.enter_context(tc.tile_pool(name="ybfp", bufs=4))
            zsbp = ctx.enter_context(tc.tile_pool(name="zsbp", bufs=4))

            # ---- constants -------------------------------------------------
            ident_b = consts.tile([128, 128], BF16)
            nc.sync.dma_start(out=ident_b, in_=idb_d[:, :])
            ident_f = consts.tile([128, 128], FP32)
            nc.sync.dma_start(out=ident_f, in_=idf_d[:, :])
            cst_t = consts.tile([NBANDS, 2], FP32)
            nc.sync.dma_start(out=cst_t, in_=cst_d[:, :])
            eps_t = consts.tile([NBANDS, 1], FP32)
            nc.vector.memset(eps_t, EPS)

            # ---- weight + indicator loads ---------------------------------
            w_tiles = []
            for b in range(NBANDS):
                tiles_b = []
                for ci, (k0, k1, last) in enumerate(BAND_CHUNKS[b]):
                    off, rows = w_offs[b][ci]
                    if b == NBANDS - 1:
                        wt_t = consts.tile([rows, EMB], F32R, name="w4")
                        nc.sync.dma_start(out=wt_t, in_=wt4_d[:, :].bitcast(F32R))
                    else:
                        wt_t = wpool.tile([rows, EMB], BF16, name="w", tag="w")
                        nc.sync.dma_start(out=wt_t, in_=wt_d[off:off + rows, :])
                    tiles_b.append(wt_t)
                w_tiles.append(tiles_b)

            ind_tiles = []
            for t in range(NTILES):
                rows = min(128, NROWS - 128 * t)
                it_ = indpool.tile([rows, NBANDS], F32R, name="ind", tag="ind")
                nc.sync.dma_start(
                    out=it_, in_=ind_d[128 * t:128 * t + rows, :].bitcast(F32R))
                ind_tiles.append(it_)

            # ---- chunk tiles (allocated up front; filled by repack) --------
            ch_tiles = []
            for b in range(NBANDS):
                tiles_b = []
                for ci, (k0, k1, last) in enumerate(BAND_CHUNKS[b]):
                    rows = (k1 - k0) + (2 if last else 0)
                    if b == NBANDS - 1:
                        cht = consts.tile([rows, NT], F32R, name="ch4")
                    else:
                        cht = chpool.tile([rows, NT], BF16, name="ch", tag="ch")
                    tiles_b.append(cht)
                ch_tiles.append(tiles_b)

            # ---- phase A: x in, squares, stats matmuls ---------------------
            with tc.tile_pool(name="psq", bufs=1, space="PSUM") as psq:
                pS = psq.tile([NBANDS, NT], FP32, name="pS")
                pQ = psq.tile([NBANDS, NT], FP32, name="pQ")

                x_tiles = []
                for t in range(NTILES):
                    rows = min(128, NROWS - 128 * t)
                    xt = xpool.tile([rows, NT], F32R, name="x", tag="x")
                    src = bass.AP(tensor=x_flat.tensor, offset=128 * t * NT,
                                  ap=[[NT, rows], [1, NT]]).bitcast(F32R)
                    nc.sync.dma_start(out=xt, in_=src)
                    x_tiles.append(xt)

                for t in range(NTILES):
                    rows = x_tiles[t].shape[0]
                    x2 = x2pool.tile([128, NT], F32R, name="x2")
                    if t % 2 == 0:
                        nc.vector.tensor_mul(x2[0:rows, :], x_tiles[t], x_tiles[t])
                    else:
                        nc.scalar.activation(
                            out=x2[0:rows, :], in_=x_tiles[t],
                            func=mybir.ActivationFunctionType.Square,
                            scale=1.0)
                    nc.tensor.matmul(pS, ind_tiles[t], x_tiles[t],
                                     start=(t == 0), stop=(t == NTILES - 1))
                    nc.tensor.matmul(pQ, ind_tiles[t], x2[0:rows, :],
                                     start=(t == 0), stop=(t == NTILES - 1))

                # ---- repack: natural tiles -> band chunk tiles (bf16) ------
                for b in range(NBANDS):
                    eng = nc.sync if b == NBANDS - 1 else nc.gpsimd
                    for ci, (k0, k1, last) in enumerate(BAND_CHUNKS[b]):
                        for (ti, sp, dp, ln) in _repack_segments(b, k0, k1):
                            eng.dma_start(
                                out=ch_tiles[b][ci][dp:dp + ln, :],
                                in_=x_tiles[ti][sp:sp + ln, :])

                # ---- phase B: stats postprocess ----------------------------
                scat = stat.tile([102, NT], F32R)       # negmu @0:38, inva @64:102
                tmp_msq = stat.tile([NBANDS, NT], FP32)
                tmp_musq = stat.tile([NBANDS, NT], FP32)
                tmp_var = stat.tile([NBANDS, NT], FP32)
                a_all = stat.tile([NBANDS, NT], FP32)

                nc.vector.tensor_scalar_mul(scat[0:NBANDS, :], pS, cst_t[:, 0:1])
                nc.vector.tensor_scalar_mul(tmp_msq, pQ, cst_t[:, 1:2])
                nc.vector.tensor_mul(tmp_musq, scat[0:NBANDS, :], scat[0:NBANDS, :])
                nc.vector.tensor_sub(tmp_var, tmp_msq, tmp_musq)
                nc.scalar.activation(out=scat[64:64 + NBANDS, :], in_=tmp_var,
                                     func=mybir.ActivationFunctionType.Sqrt,
                                     bias=eps_t, scale=1.0)
                nc.vector.reciprocal(out=a_all, in_=scat[64:64 + NBANDS, :])

            # ---- band 37 exact variance (avoid msq-musq cancellation) ------
            b37 = NBANDS - 1
            x4 = stat.tile([4, NT], FP32)
            nc.sync.dma_start(out=x4, in_=bass.AP(
                tensor=x_flat.tensor, offset=(NFREQ - 1) * NT,
                ap=[[NFREQ * NT, 4], [1, NT]]))
            negmu4 = stat.tile([4, NT], FP32)
            for rr in range(4):
                nc.sync.dma_start(out=negmu4[rr:rr + 1, :],
                                  in_=scat[b37:b37 + 1, :].bitcast(FP32))
            xc4 = stat.tile([4, NT], FP32)
            nc.vector.tensor_add(xc4, x4, negmu4)
            xcsq4 = stat.tile([4, NT], F32R)
            nc.vector.tensor_mul(xcsq4, xc4, xc4)
            ones4 = stat.tile([4, 1], F32R)
            nc.sync.dma_start(out=ones4, in_=one4_d[:, :].bitcast(F32R))

            # ---- phase C ---------------------------------------------------
            with tc.tile_pool(name="pyp", bufs=3, space="PSUM") as pyp, \
                 tc.tile_pool(name="pzt", bufs=2, space="PSUM") as pzt, \
                 tc.tile_pool(name="pat", bufs=1, space="PSUM") as pat, \
                 tc.tile_pool(name="pv", bufs=1, space="PSUM") as pvp:

                p_v = pvp.tile([1, NT], FP32, name="p_v")
                nc.tensor.matmul(p_v, ones4, xcsq4, start=True, stop=True)
                inva37 = stat.tile([1, NT], F32R)
                a37 = stat.tile([1, NT], FP32)
                nc.scalar.activation(out=inva37, in_=p_v,
                                     func=mybir.ActivationFunctionType.Sqrt,
                                     bias=eps_t[0:1, 0:1], scale=0.25)
                nc.vector.reciprocal(out=a37, in_=inva37)
                nc.sync.dma_start(out=scat[64 + b37:64 + b37 + 1, :], in_=inva37)
                nc.sync.dma_start(out=a_all[b37:b37 + 1, :], in_=a37)

                # a^T via PE transposes: a_all [38, 512] -> aT_sb [128, (c,38)]
                p_aT = pat.tile([128, 4 * NBANDS], FP32, name="p_aT")
                for c in range(4):
                    nc.tensor.transpose(p_aT[:, c * NBANDS:(c + 1) * NBANDS],
                                        a_all[:, 128 * c:128 * (c + 1)],
                                        ident_f[0:NBANDS, 0:NBANDS])
                aT_sb = stat.tile([128, 4 * NBANDS], FP32)
                nc.vector.tensor_copy(aT_sb, p_aT)

                for b in range(NBANDS):
                    # scatter aug rows [negmu_b; inva_b] into last chunk tile
                    k0, k1, _ = BAND_CHUNKS[b][-1]
                    kx = k1 - k0
                    last_t = ch_tiles[b][-1]
                    src = bass.AP(tensor=scat.tensor,
                                  offset=scat.offset + b * scat.ap[0][0],
                                  ap=[[64 * scat.ap[0][0], 2], [1, NT]])
                    seng = nc.sync if b == NBANDS - 1 else nc.gpsimd
                    seng.dma_start(out=last_t[kx:kx + 2, :], in_=src)

                    # main matmuls (accumulate over chunks)
                    p_y = pyp.tile([EMB, NT], FP32, name="p_y")
                    nch = len(BAND_CHUNKS[b])
                    for ci in range(nch):
                        nc.tensor.matmul(p_y, w_tiles[b][ci], ch_tiles[b][ci],
                                         start=(ci == 0), stop=(ci == nch - 1))

                    # psum -> sbuf bf16 (ACT), then 4 PE transposes
                    ybf = ybfp.tile([EMB, NT], BF16, name="ybf")
                    nc.scalar.copy(ybf, p_y)
                    p_zT = pzt.tile([128, 4 * 128], BF16, name="p_zT")
                    for c in range(4):
                        nc.tensor.transpose(p_zT[:, 128 * c:128 * (c + 1)],
                                            ybf[:, 128 * c:128 * (c + 1)],
                                            ident_b)

                    # scale by a^T (free-dim broadcast) -> zsb bf16
                    zsb = zsbp.tile([128, 4, 128], BF16, name="zsb")
                    bcast = bass.AP(tensor=aT_sb.tensor,
                                    offset=aT_sb.offset + b,
                                    ap=[list(aT_sb.ap[0]), [NBANDS, 4], [0, 128]])
                    nc.vector.tensor_mul(
                        zsb, p_zT[:, :].rearrange("p (c m) -> p c m", c=4), bcast)

                    # out DMA (cast bf16 -> fp32), [t, emb] layout
                    dst = bass.AP(tensor=out_d[:, :, :].tensor,
                                  offset=b * NT * EMB,
                                  ap=[[EMB, 128], [128 * EMB, 4], [1, EMB]])
                    nc.gpsimd.dma_start(out=dst, in_=zsb)

    return nc


def _host_tensors(params):
    """Precompute weight/indicator/constant host arrays (fp32/bf16)."""
    ln_g = [np.asarray(a, dtype=np.float32) for a in params["ln_g"]]
    ln_b = [np.asarray(a, dtype=np.float32) for a in params["ln_b"]]
    W = [np.asarray(a, dtype=np.float32) for a in params["W"]]
    bias = [np.asarray(a, dtype=np.float32) for a in params["b"]]

    wt_rows = NROWS + 2 * NBANDS
    WT = np.zeros((wt_rows, EMB), dtype=np.float32)
    r = 0
    for b in range(NBANDS):
        Wg = W[b] * ln_g[b][None, :]          # [128, feat]
        s = Wg.sum(axis=1)                    # [128]
        bf = bias[b] + W[b] @ ln_b[b]         # [128]
        for (k0, k1, last) in BAND_CHUNKS[b]:
            WT[r:r + (k1 - k0), :] = Wg.T[k0:k1, :]
            r += k1 - k0
            if last:
                WT[r, :] = s
                WT[r + 1, :] = bf
                r += 2
    assert r == wt_rows

    IND = np.zeros((NTILES * 128, NBANDS), dtype=np.float32)
    for g in range(NROWS):
        IND[g, BAND_OF[g % NFREQ]] = 1.0

    CST = np.zeros((NBANDS, 2), dtype=np.float32)
    for b in range(NBANDS):
        feat = 4.0 * WIDTHS[b]
        CST[b, 0] = -1.0 / feat
        CST[b, 1] = 1.0 / feat

    I_BF = np.eye(128, dtype=np.float32).astype(ml_dtypes.bfloat16)
    I_F32 = np.eye(128, dtype=np.float32)
    b37 = NBANDS - 1
    Wg37 = W[b37] * ln_g[b37][None, :]
    WT4 = np.zeros((6, EMB), dtype=np.float32)
    WT4[0:4, :] = Wg37.T
    WT4[4, :] = Wg37.sum(axis=1)
    WT4[5, :] = bias[b37] + W[b37] @ ln_b[b37]
    return {
        "one4": np.ones((4, 1), dtype=np.float32),
        "wt4": WT4,
        "wt": WT.astype(ml_dtypes.bfloat16),
        "ind": IND,
        "identb": I_BF,
        "identf": I_F32,
        "cst": CST,
    }


def kernel(x, params):
    global _NC
    _install_birpatch()
    from concourse.bass_utils import run_bass_kernel_spmd

    x = np.asarray(x, dtype=np.float32)
    assert x.shape == (NB, 4, NFREQ, NT), x.shape

    if _NC is None:
        _NC = _build_nc()

    aux = _host_tensors(params)
    in_maps = [dict(aux, x=np.ascontiguousarray(x[c])) for c in range(NCORES)]
    res = run_bass_kernel_spmd(_NC, in_maps, list(range(NCORES)))
    out = np.stack([res.results[c]["out"] for c in range(NCORES)], axis=0)
    return out.astype(np.float32)
